# revision 15
# baseline (speedup 1.0000x reference)
"""Multi-head self-attention TRN2 kernel.

Problem: x[2, 2048, 1024] -> MHSA(16 heads, head_dim 64) -> [2, 2048, 1024].

Sharding: 8 cores = 2 batches x 4 head-groups (4 heads each).  Each core:
  - gets x^T [C, T] for its batch plus the W_qkv columns / W_out rows for its
    4 heads,
  - computes QKV projections, per-head attention, and a partial output
    projection out_partial[T, C] = attn_heads @ W_out[head_rows, :],
  - host sums the 4 partials per batch and adds b_out.

Dataflow is fully transposed on-chip (no transposes anywhere):
  QT[d, t] = Wq[c, d].T @ xT[c, t]          (lhsT = Wq slice, rhs = xT)
  KT[d, t] = Wk[c, d].T @ xT[c, t]
  V[t, d]  = xT[c, t].T @ Wv[c, d]  (+ rank-1 ones x bias matmul)
  sT[k, q] = KT[d, k].T @ QT[d, q]          (scores, transposed)
  wT = exp(sT / 8)                          (ACT, no max-subtraction: scores
                                             are bounded ~N(0, 3) here)
  attnT[d, q] (+ denom row) = [V | 1][k, d+1].T @ wT[k, q]   (accumulate k)
  attnT /= denom (DVE mult with DMA-broadcast reciprocal denominators)
  out[t, c] = attnT[c, t].T @ Wout[c, cout] (accumulate over 2 c-chunks)

All matmuls run as float32r (FP22 mantissa, full PE rate at N>=256).
"""

import numpy as np

B = 2
T = 2048
C = 1024
H = 16
DH = 64
NCORES = 8
HPC = 4              # heads per core
DS = HPC * DH        # 256: per-core slice of the model dim
P = 128


def _build(nc_T=T):
    import concourse.bass as bass
    import concourse.tile as tile
    from concourse import bacc, mybir

    f32 = mybir.dt.float32
    f32r = mybir.dt.float32r
    Exp = mybir.ActivationFunctionType.Exp

    Tn = nc_T
    KT_TILES = Tn // P       # k tiles (and t tiles)
    QH = Tn // 2             # q processed in two halves per head
    NO = C // P              # 8 contraction chunks over C

    nc = bacc.Bacc(trn_type="TRN2")

    xT_d = nc.dram_tensor("xT", [C, Tn], f32, kind="ExternalInput")
    wqkv_d = nc.dram_tensor("wqkv", [C, 3 * DS], f32, kind="ExternalInput")
    bq_d = nc.dram_tensor("bq", [DS], f32, kind="ExternalInput")
    bk_d = nc.dram_tensor("bk", [DS], f32, kind="ExternalInput")
    bv_d = nc.dram_tensor("bv", [DS], f32, kind="ExternalInput")
    wout_d = nc.dram_tensor("wout", [DS, C], f32, kind="ExternalInput")
    out_d = nc.dram_tensor("out", [Tn, C], f32, kind="ExternalOutput")

    xT_r = xT_d[:].rearrange("(o p) t -> p o t", p=P)
    wqkv_r = wqkv_d[:].rearrange("(o p) n -> p o n", p=P)
    wout_r = wout_d[:].rearrange("(o p) n -> p o n", p=P)
    out_ap = out_d[:]

    def q_slices(total):
        return [(q0, min(512, total - q0)) for q0 in range(0, total, 512)]

    with tile.TileContext(nc) as tc:
        with (
            tc.tile_pool(name="const", bufs=1) as const,
            tc.tile_pool(name="qkv", bufs=1) as qkv_pool,
            tc.tile_pool(name="wout_pool", bufs=1) as wout_pool,
        ):
            bq_sb = const.tile([P, 2], f32, tag="bq")
            nc.sync.dma_start(out=bq_sb, in_=bq_d[:].rearrange("(m p) -> p m", p=P))
            bk_sb = const.tile([P, 2], f32, tag="bk")
            nc.sync.dma_start(out=bk_sb, in_=bk_d[:].rearrange("(m p) -> p m", p=P))
            # V bias broadcast across all 128 partitions (DRAM zero-step AP)
            bvb = const.tile([P, DS], f32, tag="bvb")
            bv_ap = bv_d[:]
            nc.sync.dma_start(
                out=bvb,
                in_=bass.AP(
                    tensor=bv_ap.tensor, offset=bv_ap.offset, ap=[[0, P], [1, DS]]
                ),
            )

            # Wout prefetched early; lives until phase D.
            wo_sb = wout_pool.tile([P, 2, C], f32r, tag="wo")
            for cc in range(2):
                nc.sync.dma_start(out=wo_sb[:, cc], in_=wout_r[:, cc].bitcast(f32r))

            # Persistent per-head projections.
            # QT/KT: [128, 2, T]; head h lives at partitions 64*(h%2).. of
            # m-tile h//2.
            QT = qkv_pool.tile([P, 2, Tn], f32r, tag="qt")
            KT = qkv_pool.tile([P, 2, Tn], f32r, tag="kt")
            # V: [128, t-tiles, 4 heads x 65]; per 65-col head block the ones
            # column sits last, so the attnT matmul emits 64 attn rows plus a
            # softmax-denominator row at psum partition 64.
            Vt = qkv_pool.tile([P, KT_TILES, HPC * (DH + 1)], f32r, tag="v")
            vt_h = Vt[:].rearrange("p t (h e) -> p t h e", e=DH + 1)
            nc.vector.memset(vt_h[:, :, :, DH : DH + 1].bitcast(f32), 1.0)

            # ---------------- Phase A: QKV projections ----------------
            with (
                tc.tile_pool(name="a_in", bufs=1) as a_in,
                tc.tile_pool(name="a_psum", bufs=3, space="PSUM") as a_psum,
            ):
                xs = a_in.tile([P, NO, Tn], f32r, tag="xs")
                ws = a_in.tile([P, NO, 3 * DS], f32r, tag="ws")
                for o in range(NO):
                    nc.sync.dma_start(out=ws[:, o], in_=wqkv_r[:, o].bitcast(f32r))
                    nc.sync.dma_start(out=xs[:, o], in_=xT_r[:, o].bitcast(f32r))

                for woff, bias_sb, dst in ((0, bq_sb, QT), (DS, bk_sb, KT)):
                    for m in range(2):
                        for q0, qs in q_slices(Tn):
                            ps = a_psum.tile([P, 512], f32, tag="qk")
                            for o in range(NO):
                                nc.tensor.matmul(
                                    ps[:, :qs],
                                    lhsT=ws[:, o, woff + m * P : woff + (m + 1) * P],
                                    rhs=xs[:, o, q0 : q0 + qs],
                                    start=(o == 0),
                                    stop=(o == NO - 1),
                                )
                            nc.vector.tensor_scalar_add(
                                dst[:, m, q0 : q0 + qs], ps[:, :qs], bias_sb[:, m : m + 1]
                            )

                bvb_h = bvb[:].rearrange("p (h d) -> p h d", d=DH)
                for t in range(KT_TILES):
                    vp = a_psum.tile([P, DS], f32, tag="vp")
                    for o in range(NO):
                        nc.tensor.matmul(
                            vp,
                            lhsT=xs[:, o, t * P : (t + 1) * P],
                            rhs=ws[:, o, 2 * DS : 3 * DS],
                            start=(o == 0),
                            stop=(o == NO - 1),
                        )
                    vp_h = vp[:].rearrange("p (h d) -> p h d", d=DH)
                    nc.vector.tensor_add(vt_h[:, t, :, 0:DH], vp_h, bvb_h)

            # ---------------- Phase B: attention per head ----------------
            with tc.tile_pool(name="attn", bufs=1) as attn_pool:
                AT = attn_pool.tile([P, 2, Tn], f32r, tag="at")

                with (
                    tc.tile_pool(name="b_w", bufs=3) as b_w,
                    tc.tile_pool(name="b_bc", bufs=2) as b_bc,
                    tc.tile_pool(name="b_ps", bufs=2, space="PSUM") as b_ps,
                    tc.tile_pool(name="b_pa", bufs=2, space="PSUM") as b_pa,
                ):
                    for h in range(HPC):
                        m, po = h // 2, 64 * (h % 2)
                        qt_h = QT[po : po + 64, m, :]
                        kt_h = KT[po : po + 64, m, :]
                        for qh in range(2):
                            qb = qh * QH
                            pa = b_pa.tile([P, QH], f32, tag="pa")
                            # attn rows at psum partitions 0-63, denom row at 64
                            pa_mm = pa[0:65]
                            for kt in range(KT_TILES):
                                sg = b_ps.tile([P, QH], f32, tag="sg")
                                for q0, qs in q_slices(QH):
                                    nc.tensor.matmul(
                                        sg[:, q0 : q0 + qs],
                                        lhsT=kt_h[:, kt * P : (kt + 1) * P],
                                        rhs=qt_h[:, qb + q0 : qb + q0 + qs],
                                        start=True,
                                        stop=True,
                                    )
                                wg = b_w.tile([P, QH], f32r, tag="wg")
                                nc.scalar.activation(wg, sg, Exp, scale=0.125)
                                for q0, qs in q_slices(QH):
                                    nc.tensor.matmul(
                                        pa_mm[:, q0 : q0 + qs],
                                        lhsT=Vt[:, kt, 65 * h : 65 * (h + 1)],
                                        rhs=wg[:, q0 : q0 + qs],
                                        start=(kt == 0),
                                        stop=(kt == KT_TILES - 1),
                                    )
                            # normalize: reciprocal of the denom row (PSUM->
                            # SBUF, same partition), DMA-broadcast it across
                            # partitions 0-63, multiply.  Odd heads land on
                            # partitions 64-127 of AT, which the DVE cannot
                            # reach from a 0-63 input, so they bounce through
                            # SBUF staging + DMA.
                            dn = b_bc.tile([P, QH], f32, tag="dn")
                            nc.vector.reciprocal(dn[64:65], pa[64:65])
                            bc = b_bc.tile([64, QH], f32, tag="bc")
                            dbase = dn[64:65]
                            dbcast = bass.AP(
                                tensor=dbase.tensor,
                                offset=dbase.offset,
                                ap=[[dbase.ap[0][0], 1], [0, 64], [1, QH]],
                            )
                            nc.gpsimd.dma_start(
                                out=bc[:].rearrange("p (o q) -> p o q", o=1),
                                in_=dbcast,
                            )
                            if po == 0:
                                nc.vector.tensor_mul(
                                    AT[0:64, m, qb : qb + QH], pa[0:64], bc
                                )
                            else:
                                stg = b_bc.tile([64, QH], f32r, tag="stg")
                                nc.vector.tensor_mul(stg, pa[0:64], bc)
                                nc.gpsimd.dma_start(
                                    out=AT[64:128, m, qb : qb + QH], in_=stg
                                )

                # ---------------- Phase D: output projection ----------------
                with (
                    tc.tile_pool(name="d_ps", bufs=2, space="PSUM") as d_ps,
                    tc.tile_pool(name="d_out", bufs=3) as d_out,
                ):
                    for t in range(KT_TILES):
                        op = d_ps.tile([P, C], f32, tag="op")
                        for q0, qs in q_slices(C):
                            for cc in range(2):
                                nc.tensor.matmul(
                                    op[:, q0 : q0 + qs],
                                    lhsT=AT[:, cc, t * P : (t + 1) * P],
                                    rhs=wo_sb[:, cc, q0 : q0 + qs],
                                    start=(cc == 0),
                                    stop=(cc == 1),
                                )
                        ob = d_out.tile([P, C], f32, tag="ob")
                        nc.vector.tensor_copy(ob, op)
                        nc.sync.dma_start(out=out_ap[t * P : (t + 1) * P, :], in_=ob)

    nc.compile()
    return nc


_NC_CACHE = {}


def _get_nc(nc_T=T):
    if nc_T not in _NC_CACHE:
        _NC_CACHE[nc_T] = _build(nc_T)
    return _NC_CACHE[nc_T]


def make_in_maps(x, W_qkv, b_qkv, W_out):
    """Build the 8 per-core input dicts from full inputs."""
    in_maps = []
    xTs = [np.ascontiguousarray(x[b].T.astype(np.float32)) for b in range(B)]
    for core in range(NCORES):
        b, g = core // HPC, core % HPC
        hs = g * DS
        wq = W_qkv[:, hs : hs + DS]
        wk = W_qkv[:, C + hs : C + hs + DS]
        wv = W_qkv[:, 2 * C + hs : 2 * C + hs + DS]
        in_maps.append(
            {
                "xT": xTs[b],
                "wqkv": np.ascontiguousarray(
                    np.concatenate([wq, wk, wv], axis=1).astype(np.float32)
                ),
                "bq": np.ascontiguousarray(b_qkv[hs : hs + DS].astype(np.float32)),
                "bk": np.ascontiguousarray(b_qkv[C + hs : C + hs + DS].astype(np.float32)),
                "bv": np.ascontiguousarray(
                    b_qkv[2 * C + hs : 2 * C + hs + DS].astype(np.float32)
                ),
                "wout": np.ascontiguousarray(W_out[hs : hs + DS, :].astype(np.float32)),
            }
        )
    return in_maps


def combine_outputs(outs, b_out):
    """Sum the 4 per-head-group partials per batch and add the bias."""
    out = np.empty((B, T, C), np.float32)
    for b in range(B):
        acc = outs[HPC * b].astype(np.float32).copy()
        for g in range(1, HPC):
            acc += outs[HPC * b + g]
        out[b] = acc + b_out.astype(np.float32)
    return out


def kernel(x, W_qkv, b_qkv, W_out, b_out, trace=False):
    from concourse import bass_utils

    nc = _get_nc()
    in_maps = make_in_maps(
        np.asarray(x), np.asarray(W_qkv), np.asarray(b_qkv), np.asarray(W_out)
    )
    res = bass_utils.run_bass_kernel_spmd(
        nc, in_maps, core_ids=list(range(NCORES)), trace=trace
    )
    outs = [r["out"] for r in res.results]
    out = combine_outputs(outs, np.asarray(b_out))
    if trace:
        kernel.last_results = res
    return out
